# revision 5
# baseline (speedup 1.0000x reference)
"""Multi-head attention (B=8, N=1024, C=768, H=12) on 8 TRN2 NeuronCores.

Sharding: pure data-parallel over batch — core i computes batch element i
with replicated weights. No collectives.

Per-core kernel (x: [1024, 768]):
  - xT via DMA-xbar transpose (bf16 DRAM roundtrip): x loads + stores on
    the sync DMA queue, transpose-loads on the scalar HWDGE queue. All
    weight casts f32->bf16 run on the DVE; vp ones/pad memsets on gpsimd,
    so the x-cast chain is never blocked.
  - v' = [x @ w_v | ones | pad] per head: m-tiles 0-1 before pair 0, the
    rest JIT-inserted into pair 0's j-loop (PSUM "st" ring slots).
  - qkT = (x @ w_qkv[:, :1536]).T per pair, computed as [128, 512]
    half-tiles so each holds a PSUM ring buffer ~2us: pair p+1's qkT is
    inserted into pair p's j-loop instead of stalling at the boundary.
  - ST: the two heads of a pair run as concurrent 64-contraction matmuls
    in disjoint PE row-groups (tile_position (0,0) / (64,0)) — k_a lives
    in partitions 0:64, k_b in 64:128, q likewise; no zero padding.
  - E = exp(ST/8) on ACT (the pair-loop bottleneck: 16 x 1.34us per pair),
    U' = v'^T E accumulated in PSUM with the ones-column giving the
    softmax denominator in row 64. UT trails ST/exp by one j.
  - O = U[0:64]/r (approx-reciprocal + gpsimd broadcast + DVE mul),
    out = OT.T @ w_proj + b_proj per i-half right after the final
    normalize of that half.

rel err ~5e-3 vs f32 reference (bf16 compute, f32 accumulation).
"""

import functools

import numpy as np

import concourse.bass as bass
import concourse.mybir as mybir
from concourse import bacc
from concourse.tile import TileContext
from concourse.bass_utils import run_bass_kernel_spmd

B, N, C, H = 8, 1024, 768, 12
D = C // H  # 64
SCALE = float(D) ** -0.5
F32 = mybir.dt.float32
BF16 = mybir.dt.bfloat16

KT = C // 128      # 6  contraction tiles over channels
MT = N // 128      # 8  token tiles
PAIRS = H // 2     # 6  head pairs


def _build():
    nc = bacc.Bacc(None, target_bir_lowering=False, debug=False)
    x_ext = nc.declare_dram_parameter("x", [N, C], F32, isOutput=False)
    wqkv_ext = nc.declare_dram_parameter("w_qkv", [C, 3 * C], F32, isOutput=False)
    wproj_ext = nc.declare_dram_parameter("w_proj", [C, C], F32, isOutput=False)
    bias_ext = nc.declare_dram_parameter("b_proj", [C], F32, isOutput=False)
    out_ext = nc.declare_dram_parameter("out", [N, C], F32, isOutput=True)

    with TileContext(nc) as tc:
        with (
            tc.tile_pool(name="singles", bufs=1) as singles,
            tc.tile_pool(name="stage", bufs=5) as stage,
            tc.tile_pool(name="xbf", bufs=2) as xbfp,
            tc.tile_pool(name="xt", bufs=1) as xtp,
            tc.tile_pool(name="qkt", bufs=2) as qktp,
            tc.tile_pool(name="vp", bufs=MT) as vpp,
            tc.tile_pool(name="et", bufs=4) as etp,
            tc.tile_pool(name="u", bufs=2 * PAIRS) as up,
            tc.tile_pool(name="small", bufs=3) as smallp,
            tc.tile_pool(name="outp", bufs=2) as outp,
            tc.tile_pool(name="dram", bufs=1, space="DRAM") as dramp,
            tc.tile_pool(name="ps", bufs=2, space="PSUM") as ps,
        ):
            # ---- x: load + store (sync q), cast on DVE, xbar transpose
            # (scalar HWDGE q) ----
            xdram = dramp.tile([N, C], BF16)
            for m in range(MT):
                st_x = stage.tile([128, C], F32, tag="stx")
                nc.sync.dma_start(out=st_x, in_=x_ext[m * 128:(m + 1) * 128, :])
                xb = xbfp.tile([128, C], BF16, tag="xbf")
                nc.vector.tensor_copy(out=xb, in_=st_x)
                nc.sync.dma_start(out=xdram[m * 128:(m + 1) * 128, :], in_=xb)
            xt = [xtp.tile([128, N], BF16, tag=f"xt{k}", name=f"xt{k}")
                  for k in range(KT)]
            for k in range(KT):
                nc.scalar.dma_start_transpose(
                    xt[k], xdram[:, k * 128:(k + 1) * 128])

            # ---- w_v (sync q, casts on DVE) ----
            wv = []     # 6 x [128, 768]   rhs for v
            for k in range(KT):
                st_v = stage.tile([128, C], F32, tag="stage", name=f"stv{k}")
                nc.sync.dma_start(
                    out=st_v,
                    in_=wqkv_ext[k * 128:(k + 1) * 128, 2 * C:3 * C])
                t_v = singles.tile([128, C], BF16, tag=f"wv{k}", name=f"wv{k}")
                nc.vector.tensor_copy(out=t_v, in_=st_v)
                wv.append(t_v)

            # ---- w_qk: all three 512-col chunks on sync q, casts on DVE ----
            wqk = []
            for k in range(KT):
                st_qk = stage.tile([128, 2 * C], F32, tag="stage",
                                   name=f"stqk{k}")
                t_qk = singles.tile([128, 2 * C], BF16, tag=f"wqk{k}",
                                    name=f"wqk{k}")
                for ch in range(3):
                    sl = slice(ch * 512, (ch + 1) * 512)
                    nc.sync.dma_start(
                        out=st_qk[:, sl],
                        in_=wqkv_ext[k * 128:(k + 1) * 128,
                                     ch * 512:(ch + 1) * 512])
                    nc.vector.tensor_copy(out=t_qk[:, sl], in_=st_qk[:, sl])
                wqk.append(t_qk)

            # ---- w_proj / bias on the scalar q; casts on DVE ----
            wpr = []
            for k in range(KT):
                st_pr = stage.tile([128, C], F32, tag="stage", name=f"stpr{k}")
                nc.scalar.dma_start(
                    out=st_pr,
                    in_=wproj_ext[k * 128:(k + 1) * 128, :])
                t_pr = singles.tile([128, C], BF16, tag=f"wpr{k}",
                                    name=f"wpr{k}")
                nc.vector.tensor_copy(out=t_pr, in_=st_pr)
                wpr.append(t_pr)
            bias_bc = singles.tile([128, C], F32, name="bias_bc")
            nc.scalar.dma_start(out=bias_bc,
                                in_=bias_ext[:].partition_broadcast(128))

            # ---- v' tiles: ones/pad memsets on gpsimd (doesn't block DVE) ----
            vp = []
            for m in range(MT):
                t_vp = vpp.tile([128, H, 128], BF16, tag="vp", name=f"vp{m}")
                nc.gpsimd.memset(t_vp[:, :, D:D + 1], 1.0)
                nc.gpsimd.memset(t_vp[:, :, D + 1:128], 0.0)
                vp.append(t_vp)

            def emit_vprime(m, tag):
                """v'[m] = x[m-tile] @ w_v -> vp[m][:, :, 0:64]."""
                pv = ps.tile([128, N], F32, tag=tag, name=f"pv{m}",
                             bufs=1 if tag != "st" else 2)
                for k in range(KT):
                    lhsT = xt[k][:, m * 128:(m + 1) * 128]
                    nc.tensor.matmul(pv[:, 0:512], lhsT, wv[k][:, 0:512],
                                     start=(k == 0), stop=(k == KT - 1))
                    nc.tensor.matmul(pv[:, 512:768], lhsT, wv[k][:, 512:768],
                                     start=(k == 0), stop=(k == KT - 1))
                nc.vector.tensor_copy(
                    out=vp[m][:, :, 0:D],
                    in_=pv[:, 0:C].rearrange("p (h d) -> p h d", h=H))

            # v'[0..1] before pair 0 (on the then-free ut banks); the rest
            # are JIT-inserted into pair 0's j-loop below.
            emit_vprime(0, "ut")
            emit_vprime(1, "ut2")

            # ---- qkT in [128, 512] halves (1 PSUM bank each, short holds) ----
            def emit_pq_half(t, half, who):
                pq = ps.tile([128, N], F32, tag="st", name=f"pq{who}_{half}")
                sl = slice(half * 512, (half + 1) * 512)
                for k in range(KT):
                    nc.tensor.matmul(pq[:, sl], wqk[k][:, t * 128:(t + 1) * 128],
                                     xt[k][:, sl],
                                     start=(k == 0), stop=(k == KT - 1))
                return pq, sl

            def emit_qt_half(p, qt_t, half):
                pq, sl = emit_pq_half(p, half, f"q{p}")
                nc.vector.tensor_copy(out=qt_t[:, sl], in_=pq[:, sl])

            def emit_k_half(p, ka_t, kb_t, half):
                pq, sl = emit_pq_half(PAIRS + p, half, f"k{p}")
                nc.vector.tensor_copy(out=ka_t[0:64, sl], in_=pq[0:64, sl])
                nc.vector.tensor_copy(out=kb_t[64:128, sl], in_=pq[64:128, sl])

            def alloc_qk(p):
                qt_t = qktp.tile([128, N], BF16, tag="qt", name=f"qt{p}")
                ka_t = qktp.tile([128, N], BF16, tag="ka", name=f"ka{p}")
                kb_t = qktp.tile([128, N], BF16, tag="kb", name=f"kb{p}")
                return qt_t, ka_t, kb_t

            def emit_qk_all(p, tiles):
                qt_t, ka_t, kb_t = tiles
                emit_qt_half(p, qt_t, 0)
                emit_qt_half(p, qt_t, 1)
                emit_k_half(p, ka_t, kb_t, 0)
                emit_k_half(p, ka_t, kb_t, 1)

            pending = alloc_qk(0)
            emit_qk_all(0, pending)

            upairs = {}  # (pair, ihalf) -> [128, 512] bf16 OT tile

            for p in range(PAIRS):
                qtile, ktile_a, ktile_b = pending
                if p + 1 < PAIRS:
                    nxt = alloc_qk(p + 1)

                # in-loop insertions: pair 0 gets JIT v'; later pairs get the
                # next pair's qkT halves (each holds one "st" ring buffer
                # for ~2us, under the exp cadence)
                slots = {}
                if p == 0:
                    for j, m in zip(range(1, 7), range(2, MT)):
                        slots[j] = (lambda m=m: emit_vprime(m, "st"))
                elif p + 1 < PAIRS:
                    q_t, ka_n, kb_n = nxt
                    slots[1] = lambda: emit_qt_half(p + 1, q_t, 0)
                    slots[3] = lambda: emit_qt_half(p + 1, q_t, 1)
                    slots[5] = lambda: emit_k_half(p + 1, ka_n, kb_n, 0)
                    slots[7] = lambda: emit_k_half(p + 1, ka_n, kb_n, 1)

                ut_a = ps.tile([128, N], F32, tag="ut", bufs=1, name=f"uta{p}")
                ut_b = ps.tile([128, N], F32, tag="ut2", bufs=1, name=f"utb{p}")

                ets = []  # (et_a, et_b) per j

                def emit_ut(j, ets=ets, ut_a=ut_a, ut_b=ut_b, p=p):
                    et_a, et_b = ets[j]
                    for (ut, et, h) in ((ut_a, et_a, 2 * p), (ut_b, et_b, 2 * p + 1)):
                        for ih in range(2):
                            sl = slice(ih * 512, (ih + 1) * 512)
                            nc.tensor.matmul(ut[:, sl], vp[j][:, h, :],
                                             et[:, sl],
                                             start=(j == 0), stop=(j == MT - 1))

                for j in range(MT):
                    st_a = ps.tile([128, N], F32, tag="st", name=f"sta{p}_{j}")
                    st_b = ps.tile([128, N], F32, tag="st", name=f"stb{p}_{j}")
                    ka = ktile_a[0:64, j * 128:(j + 1) * 128]
                    kb = ktile_b[64:128, j * 128:(j + 1) * 128]
                    # two concurrent 64-contraction matmuls in disjoint PE
                    # row groups: head a rows 0:63, head b rows 64:127
                    for ih in range(2):
                        sl = slice(ih * 512, (ih + 1) * 512)
                        nc.tensor.matmul(st_a[:, sl], ka, qtile[0:64, sl],
                                         start=True, stop=True)
                        nc.tensor.matmul(st_b[:, sl], kb, qtile[64:128, sl],
                                         start=True, stop=True)
                    et_a = etp.tile([128, N], BF16, tag="et", name=f"eta{p}_{j}")
                    et_b = etp.tile([128, N], BF16, tag="et", name=f"etb{p}_{j}")
                    nc.scalar.activation(
                        out=et_a, in_=st_a,
                        func=mybir.ActivationFunctionType.Exp, scale=SCALE)
                    nc.scalar.activation(
                        out=et_b, in_=st_b,
                        func=mybir.ActivationFunctionType.Exp, scale=SCALE)
                    ets.append((et_a, et_b))
                    # software-pipeline: consume last j's E while this j's exp runs
                    if j > 0:
                        emit_ut(j - 1)
                    if j in slots:
                        slots[j]()
                emit_ut(MT - 1)

                # pair 0 boundary: next pair's qkT was not in-loop (slots held
                # the JIT v'), so emit it here
                if p == 0:
                    emit_qk_all(1, nxt)
                if p + 1 < PAIRS:
                    pending = nxt

                # normalize: O = U[0:64] / r, packed [128, 512] per i-half
                for ih in range(2):
                    sl = slice(ih * 512, (ih + 1) * 512)
                    t_u = up.tile([128, 512], BF16, tag="u", name=f"u{p}_{ih}")
                    for hh, ut in ((0, ut_a), (1, ut_b)):
                        r_sb = smallp.tile([1, 512], F32, tag="rsb")
                        nc.vector.tensor_copy(out=r_sb, in_=ut[D:D + 1, sl])
                        rinv = smallp.tile([1, 512], F32, tag="rinv")
                        nc.vector.reciprocal_approx_fast(out=rinv, in_=r_sb)
                        rb = smallp.tile([64, 512], F32, tag="rb")
                        nc.gpsimd.partition_broadcast(rb, rinv)
                        nc.vector.tensor_mul(
                            out=t_u[hh * 64:(hh + 1) * 64, :],
                            in0=ut[0:D, sl], in1=rb)
                    upairs[(p, ih)] = t_u

                    # tail overlap: proj for this i-half right after the final
                    # pair's normalize of the same half
                    if p == PAIRS - 1:
                        for m in range(ih * 4, ih * 4 + 4):
                            pp = ps.tile([128, N], F32, tag="st")
                            off = (m % 4) * 128
                            for pr in range(PAIRS):
                                lhsT = upairs[(pr, ih)][:, off:off + 128]
                                nc.tensor.matmul(pp[:, 0:512], lhsT,
                                                 wpr[pr][:, 0:512],
                                                 start=(pr == 0),
                                                 stop=(pr == PAIRS - 1))
                                nc.tensor.matmul(pp[:, 512:768], lhsT,
                                                 wpr[pr][:, 512:768],
                                                 start=(pr == 0),
                                                 stop=(pr == PAIRS - 1))
                            t_o = outp.tile([128, C], F32, tag="out")
                            nc.vector.tensor_add(out=t_o, in0=pp[:, 0:C],
                                                 in1=bias_bc)
                            nc.sync.dma_start(
                                out=out_ext[m * 128:(m + 1) * 128, :], in_=t_o)

    nc.compile()
    return nc


@functools.cache
def _built():
    return _build()


def _run(inputs, trace=False, trace_cores=None):
    nc = _built()
    x = np.ascontiguousarray(np.asarray(inputs["x"], dtype=np.float32))
    w_qkv = np.ascontiguousarray(np.asarray(inputs["w_qkv"], dtype=np.float32))
    w_proj = np.ascontiguousarray(np.asarray(inputs["w_proj"], dtype=np.float32))
    b_proj = np.ascontiguousarray(np.asarray(inputs["b_proj"], dtype=np.float32))
    in_maps = [
        {"x": x[i], "w_qkv": w_qkv, "w_proj": w_proj, "b_proj": b_proj}
        for i in range(B)
    ]
    res = run_bass_kernel_spmd(
        nc, in_maps, core_ids=list(range(B)), trace=trace,
        trace_cores=trace_cores,
    )
    out = np.stack([res.results[i]["out"] for i in range(B)], axis=0)
    return out, res


def kernel(**inputs) -> np.ndarray:
    out, _ = _run(inputs, trace=False)
    return out


# revision 6
# speedup vs baseline: 1.0910x; 1.0910x over previous
"""Multi-head attention (B=8, N=1024, C=768, H=12) on 8 TRN2 NeuronCores.

Sharding: pure data-parallel over batch — core i computes batch element i
with replicated weights. No collectives.

Per-core kernel (x: [1024, 768]):
  - xT via DMA-xbar transpose (bf16 DRAM roundtrip): x loads + stores on
    the sync DMA queue, transpose-loads + w_proj/bias on the scalar HWDGE
    queue. Engine balance: x/wv/wqk01 casts on DVE (x casts high
    priority), wqk-ch2/wproj casts + vp memsets on gpsimd, so neither the
    DVE nor the ACT engine blocks the startup chain.
  - v' = [x @ w_v | ones | pad] per head: m-tiles 0-1 before pair 0, the
    rest JIT-inserted into pair 0's j-loop (PSUM "st" ring slots).
  - qkT per pair as [128, 512] half-tiles (short PSUM ring holds); pair
    p+1's qkT runs inside pair p's j-loop instead of at the boundary.
    k_a/k_b live in one [128, N] tile (k_a rows 0:64, k_b rows 64:128).
  - ST: the two heads run as 64-contraction matmuls in disjoint PE
    row-groups (tile_position (0,0)/(64,0)); no zero padding.
  - E = exp(ST/8) on ACT — the pair-loop floor (16 x ~1.11us per pair).
    U' = v'^T E accumulated in PSUM, ones-column -> denominator row 64.
  - normalize batched per pair: r copy [1,1024] + approx-reciprocal +
    gpsimd broadcast [64,1024] + DVE mul per head -> u[p] [128, 1024].
  - out = u^T @ w_proj + b_proj at the tail, per m-tile with DMA overlap.

rel err ~5e-3 vs f32 reference (bf16 compute, f32 accumulation).
"""

import functools

import numpy as np

import concourse.bass as bass
import concourse.mybir as mybir
from concourse import bacc
from concourse.tile import TileContext
from concourse.bass_utils import run_bass_kernel_spmd

B, N, C, H = 8, 1024, 768, 12
D = C // H  # 64
SCALE = float(D) ** -0.5
F32 = mybir.dt.float32
BF16 = mybir.dt.bfloat16

KT = C // 128      # 6  contraction tiles over channels
MT = N // 128      # 8  token tiles
PAIRS = H // 2     # 6  head pairs


def _build():
    nc = bacc.Bacc(None, target_bir_lowering=False, debug=False)
    x_ext = nc.declare_dram_parameter("x", [N, C], F32, isOutput=False)
    wqkv_ext = nc.declare_dram_parameter("w_qkv", [C, 3 * C], F32, isOutput=False)
    wproj_ext = nc.declare_dram_parameter("w_proj", [C, C], F32, isOutput=False)
    bias_ext = nc.declare_dram_parameter("b_proj", [C], F32, isOutput=False)
    out_ext = nc.declare_dram_parameter("out", [N, C], F32, isOutput=True)

    with TileContext(nc) as tc:
        with (
            tc.tile_pool(name="singles", bufs=1) as singles,
            tc.tile_pool(name="stage", bufs=5) as stage,
            tc.tile_pool(name="xbf", bufs=2) as xbfp,
            tc.tile_pool(name="xt", bufs=1) as xtp,
            tc.tile_pool(name="qkt", bufs=2) as qktp,
            tc.tile_pool(name="vp", bufs=MT) as vpp,
            tc.tile_pool(name="et", bufs=4) as etp,
            tc.tile_pool(name="u", bufs=PAIRS) as up,
            tc.tile_pool(name="small", bufs=3) as smallp,
            tc.tile_pool(name="outp", bufs=2) as outp,
            tc.tile_pool(name="dram", bufs=1, space="DRAM") as dramp,
            tc.tile_pool(name="ps", bufs=2, space="PSUM") as ps,
        ):
            # ---- x: load + store (sync q), cast on DVE (high priority),
            # xbar transpose on the scalar HWDGE q ----
            xdram = dramp.tile([N, C], BF16)
            for m in range(MT):
                st_x = stage.tile([128, C], F32, tag="stx")
                nc.sync.dma_start(out=st_x, in_=x_ext[m * 128:(m + 1) * 128, :])
                xb = xbfp.tile([128, C], BF16, tag="xbf")
                with tc.high_priority():
                    nc.vector.tensor_copy(out=xb, in_=st_x)
                nc.sync.dma_start(out=xdram[m * 128:(m + 1) * 128, :], in_=xb)
            xt = [xtp.tile([128, N], BF16, tag=f"xt{k}", name=f"xt{k}")
                  for k in range(KT)]
            for k in range(KT):
                nc.scalar.dma_start_transpose(
                    xt[k], xdram[:, k * 128:(k + 1) * 128])

            # ---- w_v (sync q, casts on DVE) ----
            wv = []     # 6 x [128, 768]   rhs for v
            for k in range(KT):
                st_v = stage.tile([128, C], F32, tag="stage", name=f"stv{k}")
                nc.sync.dma_start(
                    out=st_v,
                    in_=wqkv_ext[k * 128:(k + 1) * 128, 2 * C:3 * C])
                t_v = singles.tile([128, C], BF16, tag=f"wv{k}", name=f"wv{k}")
                nc.vector.tensor_copy(out=t_v, in_=st_v)
                wv.append(t_v)

            # ---- v' tiles: ones/pad memsets on gpsimd ----
            vp = []
            for m in range(MT):
                t_vp = vpp.tile([128, H, 128], BF16, tag="vp", name=f"vp{m}")
                nc.gpsimd.memset(t_vp[:, :, D:D + 1], 1.0)
                nc.gpsimd.memset(t_vp[:, :, D + 1:128], 0.0)
                vp.append(t_vp)

            # ---- w_qk: chunks 0,1 on sync q + DVE casts (feed early qkT);
            # chunk 2 on sync q + gpsimd cast (needed from pair 1) ----
            wqk = []
            for k in range(KT):
                st_qk = stage.tile([128, 2 * C], F32, tag="stage",
                                   name=f"stqk{k}")
                t_qk = singles.tile([128, 2 * C], BF16, tag=f"wqk{k}",
                                    name=f"wqk{k}")
                for ch in range(3):
                    sl = slice(ch * 512, (ch + 1) * 512)
                    nc.sync.dma_start(
                        out=st_qk[:, sl],
                        in_=wqkv_ext[k * 128:(k + 1) * 128,
                                     ch * 512:(ch + 1) * 512])
                    eng = nc.vector if ch < 2 else nc.gpsimd
                    eng.tensor_copy(out=t_qk[:, sl], in_=st_qk[:, sl])
                wqk.append(t_qk)

            # ---- w_proj / bias on the scalar q; casts on gpsimd ----
            wpr = []
            for k in range(KT):
                st_pr = stage.tile([128, C], F32, tag="stage", name=f"stpr{k}")
                nc.scalar.dma_start(
                    out=st_pr,
                    in_=wproj_ext[k * 128:(k + 1) * 128, :])
                t_pr = singles.tile([128, C], BF16, tag=f"wpr{k}",
                                    name=f"wpr{k}")
                nc.gpsimd.tensor_copy(out=t_pr, in_=st_pr)
                wpr.append(t_pr)
            bias_bc = singles.tile([128, C], F32, name="bias_bc")
            nc.scalar.dma_start(out=bias_bc,
                                in_=bias_ext[:].partition_broadcast(128))

            def emit_vprime(m, tag):
                """v'[m] = x[m-tile] @ w_v -> vp[m][:, :, 0:64]."""
                pv = ps.tile([128, N], F32, tag=tag, name=f"pv{m}",
                             bufs=1 if tag != "st" else 2)
                for k in range(KT):
                    lhsT = xt[k][:, m * 128:(m + 1) * 128]
                    nc.tensor.matmul(pv[:, 0:512], lhsT, wv[k][:, 0:512],
                                     start=(k == 0), stop=(k == KT - 1))
                    nc.tensor.matmul(pv[:, 512:768], lhsT, wv[k][:, 512:768],
                                     start=(k == 0), stop=(k == KT - 1))
                nc.vector.tensor_copy(
                    out=vp[m][:, :, 0:D],
                    in_=pv[:, 0:C].rearrange("p (h d) -> p h d", h=H))

            # v'[0..1] before pair 0 (on the then-free ut banks); the rest
            # are JIT-inserted into pair 0's j-loop below.
            emit_vprime(0, "ut")
            emit_vprime(1, "ut2")

            # ---- qkT in [128, 512] halves (1 PSUM bank each, short holds) ----
            def emit_pq_half(t, half, who):
                pq = ps.tile([128, N], F32, tag="st", name=f"pq{who}_{half}")
                sl = slice(half * 512, (half + 1) * 512)
                for k in range(KT):
                    nc.tensor.matmul(pq[:, sl], wqk[k][:, t * 128:(t + 1) * 128],
                                     xt[k][:, sl],
                                     start=(k == 0), stop=(k == KT - 1))
                return pq, sl

            def emit_qt_half(p, qt_t, half):
                pq, sl = emit_pq_half(p, half, f"q{p}")
                nc.vector.tensor_copy(out=qt_t[:, sl], in_=pq[:, sl])

            def emit_k_half(p, kab_t, half):
                pq, sl = emit_pq_half(PAIRS + p, half, f"k{p}")
                nc.vector.tensor_copy(out=kab_t[:, sl], in_=pq[:, sl])

            def alloc_qk(p):
                qt_t = qktp.tile([128, N], BF16, tag="qt", name=f"qt{p}")
                kab_t = qktp.tile([128, N], BF16, tag="kab", name=f"kab{p}")
                return qt_t, kab_t

            def emit_qk_all(p, tiles):
                qt_t, kab_t = tiles
                emit_qt_half(p, qt_t, 0)
                emit_qt_half(p, qt_t, 1)
                emit_k_half(p, kab_t, 0)
                emit_k_half(p, kab_t, 1)

            pending = alloc_qk(0)
            emit_qk_all(0, pending)

            upairs = {}  # pair -> [128, N] bf16 OT tile (a rows 0:64, b 64:128)

            for p in range(PAIRS):
                qtile, ktile = pending
                if p + 1 < PAIRS:
                    nxt = alloc_qk(p + 1)

                # in-loop insertions: pair 0 gets JIT v'; later pairs get the
                # next pair's qkT halves (each holds one "st" ring buffer
                # for ~2us, under the exp cadence)
                slots = {}
                if p == 0:
                    for j, m in zip(range(1, 7), range(2, MT)):
                        slots[j] = (lambda m=m: emit_vprime(m, "st"))
                elif p + 1 < PAIRS:
                    q_t, kab_n = nxt
                    slots[1] = lambda: emit_qt_half(p + 1, q_t, 0)
                    slots[3] = lambda: emit_qt_half(p + 1, q_t, 1)
                    slots[5] = lambda: emit_k_half(p + 1, kab_n, 0)
                    slots[7] = lambda: emit_k_half(p + 1, kab_n, 1)

                ut_a = ps.tile([128, N], F32, tag="ut", bufs=1, name=f"uta{p}")
                ut_b = ps.tile([128, N], F32, tag="ut2", bufs=1, name=f"utb{p}")

                ets = []  # (et_a, et_b) per j

                def emit_ut(j, ets=ets, ut_a=ut_a, ut_b=ut_b, p=p):
                    et_a, et_b = ets[j]
                    for (ut, et, h) in ((ut_a, et_a, 2 * p), (ut_b, et_b, 2 * p + 1)):
                        for ih in range(2):
                            sl = slice(ih * 512, (ih + 1) * 512)
                            nc.tensor.matmul(ut[:, sl], vp[j][:, h, :],
                                             et[:, sl],
                                             start=(j == 0), stop=(j == MT - 1))

                for j in range(MT):
                    st_a = ps.tile([128, N], F32, tag="st", name=f"sta{p}_{j}")
                    st_b = ps.tile([128, N], F32, tag="st", name=f"stb{p}_{j}")
                    ka = ktile[0:64, j * 128:(j + 1) * 128]
                    kb = ktile[64:128, j * 128:(j + 1) * 128]
                    # two 64-contraction matmuls in disjoint PE row groups:
                    # head a rows 0:63, head b rows 64:127
                    for ih in range(2):
                        sl = slice(ih * 512, (ih + 1) * 512)
                        nc.tensor.matmul(st_a[:, sl], ka, qtile[0:64, sl],
                                         start=True, stop=True)
                        nc.tensor.matmul(st_b[:, sl], kb, qtile[64:128, sl],
                                         start=True, stop=True)
                    et_a = etp.tile([128, N], BF16, tag="et", name=f"eta{p}_{j}")
                    et_b = etp.tile([128, N], BF16, tag="et", name=f"etb{p}_{j}")
                    nc.scalar.activation(
                        out=et_a, in_=st_a,
                        func=mybir.ActivationFunctionType.Exp, scale=SCALE)
                    nc.scalar.activation(
                        out=et_b, in_=st_b,
                        func=mybir.ActivationFunctionType.Exp, scale=SCALE)
                    ets.append((et_a, et_b))
                    # software-pipeline: consume last j's E while this j's exp runs
                    if j > 0:
                        emit_ut(j - 1)
                    if j in slots:
                        slots[j]()
                emit_ut(MT - 1)

                # pair 0 boundary: next pair's qkT was not in-loop (slots held
                # the JIT v'), so emit it here
                if p == 0:
                    emit_qk_all(1, nxt)
                if p + 1 < PAIRS:
                    pending = nxt

                # normalize, batched per pair: O = U[0:64] / r
                t_u = up.tile([128, N], BF16, tag="u", name=f"u{p}")
                for hh, ut in ((0, ut_a), (1, ut_b)):
                    r_sb = smallp.tile([1, N], F32, tag="rsb")
                    nc.vector.tensor_copy(out=r_sb, in_=ut[D:D + 1, :])
                    rinv = smallp.tile([1, N], F32, tag="rinv")
                    nc.vector.reciprocal_approx_fast(out=rinv, in_=r_sb)
                    rb = smallp.tile([64, N], F32, tag="rb")
                    nc.gpsimd.partition_broadcast(rb, rinv)
                    nc.vector.tensor_mul(
                        out=t_u[hh * 64:(hh + 1) * 64, :],
                        in0=ut[0:D, :], in1=rb)
                upairs[p] = t_u

            # ---- proj + bias tail, per m-tile with DMA overlap ----
            for m in range(MT):
                pp = ps.tile([128, N], F32, tag="st")
                cols = slice(m * 128, (m + 1) * 128)
                for pr in range(PAIRS):
                    lhsT = upairs[pr][:, cols]
                    nc.tensor.matmul(pp[:, 0:512], lhsT, wpr[pr][:, 0:512],
                                     start=(pr == 0), stop=(pr == PAIRS - 1))
                    nc.tensor.matmul(pp[:, 512:768], lhsT, wpr[pr][:, 512:768],
                                     start=(pr == 0), stop=(pr == PAIRS - 1))
                t_o = outp.tile([128, C], F32, tag="out")
                nc.vector.tensor_add(out=t_o, in0=pp[:, 0:C], in1=bias_bc)
                nc.sync.dma_start(out=out_ext[m * 128:(m + 1) * 128, :], in_=t_o)

    nc.compile()
    return nc


@functools.cache
def _built():
    return _build()


def _run(inputs, trace=False, trace_cores=None):
    nc = _built()
    x = np.ascontiguousarray(np.asarray(inputs["x"], dtype=np.float32))
    w_qkv = np.ascontiguousarray(np.asarray(inputs["w_qkv"], dtype=np.float32))
    w_proj = np.ascontiguousarray(np.asarray(inputs["w_proj"], dtype=np.float32))
    b_proj = np.ascontiguousarray(np.asarray(inputs["b_proj"], dtype=np.float32))
    in_maps = [
        {"x": x[i], "w_qkv": w_qkv, "w_proj": w_proj, "b_proj": b_proj}
        for i in range(B)
    ]
    res = run_bass_kernel_spmd(
        nc, in_maps, core_ids=list(range(B)), trace=trace,
        trace_cores=trace_cores,
    )
    out = np.stack([res.results[i]["out"] for i in range(B)], axis=0)
    return out, res


def kernel(**inputs) -> np.ndarray:
    out, _ = _run(inputs, trace=False)
    return out


# revision 11
# speedup vs baseline: 1.2623x; 1.1570x over previous
"""Multi-head attention (B=8, N=1024, C=768, H=12) on 8 TRN2 NeuronCores.

Sharding: pure data-parallel over batch — core i computes batch element i
with replicated weights. No collectives.

Per-core kernel (x: [1024, 768]):
  - xT via DMA-xbar transpose (bf16 DRAM roundtrip): x loads + stores on
    the sync DMA queue, transpose-loads + w_proj/bias on the scalar HWDGE
    queue. Engine balance: x/wv/wqk01 casts on DVE (x casts high
    priority), wqk-ch2/wproj casts + vp memsets on gpsimd, so neither the
    DVE nor the ACT engine blocks the startup chain.
  - v' = [x @ w_v | ones | pad] per head: m-tiles 0-1 before pair 0, the
    rest JIT-inserted into pair 0's j-loop (PSUM "st" ring slots).
  - qkT per pair as [128, 512] half-tiles (short PSUM ring holds); pair
    p+1's qkT runs inside pair p's j-loop instead of at the boundary.
    k_a/k_b live in one [128, N] tile (k_a rows 0:64, k_b rows 64:128).
  - ST: the two heads run as 64-contraction matmuls in disjoint PE
    row-groups (tile_position (0,0)/(64,0)); no zero padding.
  - E = exp(ST/8) on ACT — the pair-loop floor (16 x ~1.11us per pair).
    U' = v'^T E accumulated in PSUM, ones-column -> denominator row 64.
  - normalize batched per pair: r copy [1,1024] + approx-reciprocal +
    gpsimd broadcast [64,1024] + DVE mul per head -> u[p] [128, 1024].
  - out = u^T @ w_proj + b_proj at the tail, per m-tile with DMA overlap.

rel err ~5e-3 vs f32 reference (bf16 compute, f32 accumulation).
"""

import functools

import numpy as np

import concourse.bass as bass
import concourse.mybir as mybir
from concourse import bacc
from concourse.masks import make_identity
from concourse.tile import TileContext
from concourse.bass_utils import run_bass_kernel_spmd

B, N, C, H = 8, 1024, 768, 12
D = C // H  # 64
SCALE = float(D) ** -0.5
F32 = mybir.dt.float32
BF16 = mybir.dt.bfloat16

KT = C // 128      # 6  contraction tiles over channels
MT = N // 128      # 8  token tiles
PAIRS = H // 2     # 6  head pairs


def _build():
    nc = bacc.Bacc(None, target_bir_lowering=False, debug=False)
    x_ext = nc.declare_dram_parameter("x", [N, C], F32, isOutput=False)
    wqkv_ext = nc.declare_dram_parameter("w_qkv", [C, 3 * C], F32, isOutput=False)
    wproj_ext = nc.declare_dram_parameter("w_proj", [C, C], F32, isOutput=False)
    bias_ext = nc.declare_dram_parameter("b_proj", [C], F32, isOutput=False)
    out_ext = nc.declare_dram_parameter("out", [N, C], F32, isOutput=True)

    with TileContext(nc) as tc:
        with (
            tc.tile_pool(name="singles", bufs=1) as singles,
            tc.tile_pool(name="stage", bufs=5) as stage,
            tc.tile_pool(name="xbf", bufs=2) as xbfp,
            tc.tile_pool(name="xt", bufs=1) as xtp,
            tc.tile_pool(name="qkt", bufs=2) as qktp,
            tc.tile_pool(name="vp", bufs=MT) as vpp,
            tc.tile_pool(name="et", bufs=4) as etp,
            tc.tile_pool(name="u", bufs=PAIRS) as up,
            tc.tile_pool(name="small", bufs=3) as smallp,
            tc.tile_pool(name="outp", bufs=2) as outp,
            tc.tile_pool(name="dram", bufs=1, space="DRAM") as dramp,
            tc.tile_pool(name="ps", bufs=2, space="PSUM") as ps,
        ):
            # ---- x: load (sync q) then PE-transpose per m-tile into xt.
            # No DRAM roundtrip: transpose chunks land in a 1.5-bank PSUM
            # region, one DVE copy per m casts f32->bf16 into xt. ----
            ident = singles.tile([128, 128], BF16, name="ident")
            make_identity(nc, ident)
            xt_all = xtp.tile([128, KT, N], BF16, name="xt_all")
            xt = [xt_all[:, k, :] for k in range(KT)]
            for m in range(MT):
                st_x = stage.tile([128, C], F32, tag="stx")
                nc.sync.dma_start(out=st_x, in_=x_ext[m * 128:(m + 1) * 128, :])
                xb = xbfp.tile([128, C], BF16, tag="xbf")
                with tc.high_priority():
                    nc.vector.tensor_copy(out=xb, in_=st_x)
                tr = ps.tile([128, 2 * N], BF16, tag="st", name=f"tr{m}")
                for k in range(KT):
                    nc.tensor.transpose(
                        tr[:, k * 128:(k + 1) * 128],
                        xb[:, k * 128:(k + 1) * 128], ident)
                with tc.high_priority():
                    nc.vector.tensor_copy(
                        out=xt_all[:, :, m * 128:(m + 1) * 128],
                        in_=tr[:, 0:C].rearrange("p (k t) -> p k t", k=KT))

            # ---- w_v (sync q, casts on DVE) ----
            wv = []     # 6 x [128, 768]   rhs for v
            for k in range(KT):
                st_v = stage.tile([128, C], F32, tag="stage", name=f"stv{k}")
                nc.sync.dma_start(
                    out=st_v,
                    in_=wqkv_ext[k * 128:(k + 1) * 128, 2 * C:3 * C])
                t_v = singles.tile([128, C], BF16, tag=f"wv{k}", name=f"wv{k}")
                nc.vector.tensor_copy(out=t_v, in_=st_v)
                wv.append(t_v)

            # ---- v' tiles: ones/pad memsets on gpsimd ----
            vp = []
            for m in range(MT):
                t_vp = vpp.tile([128, H, 128], BF16, tag="vp", name=f"vp{m}")
                nc.gpsimd.memset(t_vp[:, :, D:D + 1], 1.0)
                nc.gpsimd.memset(t_vp[:, :, D + 1:128], 0.0)
                vp.append(t_vp)

            # ---- w_qk: chunks 0,1 on sync q + DVE casts (feed early qkT);
            # chunk 2 on sync q + gpsimd cast (needed from pair 1) ----
            wqk = []
            for k in range(KT):
                st_qk = stage.tile([128, 2 * C], F32, tag="stage",
                                   name=f"stqk{k}")
                t_qk = singles.tile([128, 2 * C], BF16, tag=f"wqk{k}",
                                    name=f"wqk{k}")
                for ch in range(3):
                    sl = slice(ch * 512, (ch + 1) * 512)
                    nc.sync.dma_start(
                        out=st_qk[:, sl],
                        in_=wqkv_ext[k * 128:(k + 1) * 128,
                                     ch * 512:(ch + 1) * 512])
                    eng = nc.vector if ch < 2 else nc.gpsimd
                    eng.tensor_copy(out=t_qk[:, sl], in_=st_qk[:, sl])
                wqk.append(t_qk)

            # ---- w_proj / bias on the scalar q; casts on gpsimd ----
            wpr = []
            for k in range(KT):
                st_pr = stage.tile([128, C], F32, tag="stage", name=f"stpr{k}")
                nc.scalar.dma_start(
                    out=st_pr,
                    in_=wproj_ext[k * 128:(k + 1) * 128, :])
                t_pr = singles.tile([128, C], BF16, tag=f"wpr{k}",
                                    name=f"wpr{k}")
                nc.gpsimd.tensor_copy(out=t_pr, in_=st_pr)
                wpr.append(t_pr)
            bias_bc = singles.tile([128, C], F32, name="bias_bc")
            nc.scalar.dma_start(out=bias_bc,
                                in_=bias_ext[:].partition_broadcast(128))

            def emit_vprime(m, tag):
                """v'[m] = x[m-tile] @ w_v -> vp[m][:, :, 0:64]."""
                pv = ps.tile([128, N], F32, tag=tag, name=f"pv{m}",
                             bufs=1 if tag != "st" else 2)
                for k in range(KT):
                    lhsT = xt[k][:, m * 128:(m + 1) * 128]
                    nc.tensor.matmul(pv[:, 0:512], lhsT, wv[k][:, 0:512],
                                     start=(k == 0), stop=(k == KT - 1))
                    nc.tensor.matmul(pv[:, 512:768], lhsT, wv[k][:, 512:768],
                                     start=(k == 0), stop=(k == KT - 1))
                nc.vector.tensor_copy(
                    out=vp[m][:, :, 0:D],
                    in_=pv[:, 0:C].rearrange("p (h d) -> p h d", h=H))

            # v'[0..1] before pair 0 (on the then-free ut banks); the rest
            # are JIT-inserted into pair 0's j-loop below.
            emit_vprime(0, "ut")
            emit_vprime(1, "ut2")

            # ---- qkT in [128, 512] halves (1 PSUM bank each, short holds) ----
            def emit_pq_half(t, half, who):
                pq = ps.tile([128, N], F32, tag="st", name=f"pq{who}_{half}")
                sl = slice(half * 512, (half + 1) * 512)
                for k in range(KT):
                    nc.tensor.matmul(pq[:, sl], wqk[k][:, t * 128:(t + 1) * 128],
                                     xt[k][:, sl],
                                     start=(k == 0), stop=(k == KT - 1))
                return pq, sl

            def emit_qt_half(p, qt_t, half):
                pq, sl = emit_pq_half(p, half, f"q{p}")
                nc.vector.tensor_copy(out=qt_t[:, sl], in_=pq[:, sl])

            def emit_k_half(p, kab_t, half):
                pq, sl = emit_pq_half(PAIRS + p, half, f"k{p}")
                nc.vector.tensor_copy(out=kab_t[:, sl], in_=pq[:, sl])

            def alloc_qk(p):
                qt_t = qktp.tile([128, N], BF16, tag="qt", name=f"qt{p}")
                kab_t = qktp.tile([128, N], BF16, tag="kab", name=f"kab{p}")
                return qt_t, kab_t

            def emit_qk_all(p, tiles):
                qt_t, kab_t = tiles
                emit_qt_half(p, qt_t, 0)
                emit_qt_half(p, qt_t, 1)
                emit_k_half(p, kab_t, 0)
                emit_k_half(p, kab_t, 1)

            pending = alloc_qk(0)
            emit_qk_all(0, pending)

            upairs = {}  # pair -> [128, N] bf16 OT tile (a rows 0:64, b 64:128)

            for p in range(PAIRS):
                qtile, ktile = pending
                if p + 1 < PAIRS:
                    nxt = alloc_qk(p + 1)

                # in-loop insertions: pair 0 gets JIT v'; later pairs get the
                # next pair's qkT halves (each holds one "st" ring buffer
                # for ~2us, under the exp cadence)
                slots = {}
                if p == 0:
                    for j, m in zip(range(1, 7), range(2, MT)):
                        slots[j] = (lambda m=m: emit_vprime(m, "st"))
                elif p + 1 < PAIRS:
                    q_t, kab_n = nxt
                    slots[1] = lambda: emit_qt_half(p + 1, q_t, 0)
                    slots[3] = lambda: emit_qt_half(p + 1, q_t, 1)
                    slots[5] = lambda: emit_k_half(p + 1, kab_n, 0)
                    slots[7] = lambda: emit_k_half(p + 1, kab_n, 1)

                ut_a = ps.tile([128, N], F32, tag="ut", bufs=1, name=f"uta{p}")
                ut_b = ps.tile([128, N], F32, tag="ut2", bufs=1, name=f"utb{p}")

                ets = []  # (et_a, et_b) per j

                def emit_ut(j, ets=ets, ut_a=ut_a, ut_b=ut_b, p=p):
                    et_a, et_b = ets[j]
                    for (ut, et, h) in ((ut_a, et_a, 2 * p), (ut_b, et_b, 2 * p + 1)):
                        for ih in range(2):
                            sl = slice(ih * 512, (ih + 1) * 512)
                            nc.tensor.matmul(ut[:, sl], vp[j][:, h, :],
                                             et[:, sl],
                                             start=(j == 0), stop=(j == MT - 1))

                for j in range(MT):
                    st_a = ps.tile([128, N], F32, tag="st", name=f"sta{p}_{j}")
                    st_b = ps.tile([128, N], F32, tag="st", name=f"stb{p}_{j}")
                    ka = ktile[0:64, j * 128:(j + 1) * 128]
                    kb = ktile[64:128, j * 128:(j + 1) * 128]
                    # two 64-contraction matmuls in disjoint PE row groups:
                    # head a rows 0:63, head b rows 64:127
                    for ih in range(2):
                        sl = slice(ih * 512, (ih + 1) * 512)
                        nc.tensor.matmul(st_a[:, sl], ka, qtile[0:64, sl],
                                         start=True, stop=True)
                        nc.tensor.matmul(st_b[:, sl], kb, qtile[64:128, sl],
                                         start=True, stop=True)
                    et_a = etp.tile([128, N], BF16, tag="et", name=f"eta{p}_{j}")
                    et_b = etp.tile([128, N], BF16, tag="et", name=f"etb{p}_{j}")
                    nc.scalar.activation(
                        out=et_a, in_=st_a,
                        func=mybir.ActivationFunctionType.Exp, scale=SCALE)
                    nc.scalar.activation(
                        out=et_b, in_=st_b,
                        func=mybir.ActivationFunctionType.Exp, scale=SCALE)
                    ets.append((et_a, et_b))
                    # software-pipeline: consume last j's E while this j's exp runs
                    if j > 0:
                        emit_ut(j - 1)
                    if j in slots:
                        slots[j]()
                emit_ut(MT - 1)

                # pair 0 boundary: next pair's qkT was not in-loop (slots held
                # the JIT v'), so emit it here
                if p == 0:
                    emit_qk_all(1, nxt)
                if p + 1 < PAIRS:
                    pending = nxt

                # normalize, batched per pair: O = U[0:64] / r
                t_u = up.tile([128, N], BF16, tag="u", name=f"u{p}")
                for hh, ut in ((0, ut_a), (1, ut_b)):
                    r_sb = smallp.tile([1, N], F32, tag="rsb")
                    nc.vector.tensor_copy(out=r_sb, in_=ut[D:D + 1, :])
                    rinv = smallp.tile([1, N], F32, tag="rinv")
                    nc.vector.reciprocal_approx_fast(out=rinv, in_=r_sb)
                    rb = smallp.tile([64, N], F32, tag="rb")
                    nc.gpsimd.partition_broadcast(rb, rinv)
                    nc.vector.tensor_mul(
                        out=t_u[hh * 64:(hh + 1) * 64, :],
                        in0=ut[0:D, :], in1=rb)
                upairs[p] = t_u

            # ---- proj + bias tail, per m-tile with DMA overlap ----
            for m in range(MT):
                pp = ps.tile([128, N], F32, tag="st")
                cols = slice(m * 128, (m + 1) * 128)
                for pr in range(PAIRS):
                    lhsT = upairs[pr][:, cols]
                    nc.tensor.matmul(pp[:, 0:512], lhsT, wpr[pr][:, 0:512],
                                     start=(pr == 0), stop=(pr == PAIRS - 1))
                    nc.tensor.matmul(pp[:, 512:768], lhsT, wpr[pr][:, 512:768],
                                     start=(pr == 0), stop=(pr == PAIRS - 1))
                t_o = outp.tile([128, C], F32, tag="out")
                nc.vector.tensor_add(out=t_o, in0=pp[:, 0:C], in1=bias_bc)
                nc.sync.dma_start(out=out_ext[m * 128:(m + 1) * 128, :], in_=t_o)

    nc.compile()
    return nc


@functools.cache
def _built():
    return _build()


def _run(inputs, trace=False, trace_cores=None):
    nc = _built()
    x = np.ascontiguousarray(np.asarray(inputs["x"], dtype=np.float32))
    w_qkv = np.ascontiguousarray(np.asarray(inputs["w_qkv"], dtype=np.float32))
    w_proj = np.ascontiguousarray(np.asarray(inputs["w_proj"], dtype=np.float32))
    b_proj = np.ascontiguousarray(np.asarray(inputs["b_proj"], dtype=np.float32))
    in_maps = [
        {"x": x[i], "w_qkv": w_qkv, "w_proj": w_proj, "b_proj": b_proj}
        for i in range(B)
    ]
    res = run_bass_kernel_spmd(
        nc, in_maps, core_ids=list(range(B)), trace=trace,
        trace_cores=trace_cores,
    )
    out = np.stack([res.results[i]["out"] for i in range(B)], axis=0)
    return out, res


def kernel(**inputs) -> np.ndarray:
    out, _ = _run(inputs, trace=False)
    return out
